# revision 1
# baseline (speedup 1.0000x reference)
"""Trainium2 Bass kernel for nn_GRUModel (segment-GRU encoder + 1-step GRU decoder).

Sharding: data-parallel over batch B: 8 cores x 16 batches each
(rows n = b_loc*64 + c, R=1024 rows/core). Weights replicated.

Layout: fully transposed. State hT is [D(partitions), rows(free)] so the
recurrent matmul ghT = Whh @ hT consumes exactly what the elementwise update
produces -- no transposes anywhere. Gates accumulate x-side and h-side into
the same PSUM bank. All matmuls in bf16 (1 cyc/row on PE vs 4 for fp32);
elementwise in bf16 where SBUF-only (DVE 2x), fp32 through PSUM.

seq_last handling:
  - encoder: emb = silu((x - last) @ W^T + b) folded into a K=65 matmul
    (extra contraction row carrying -rowsum(W_emb) * last).
  - output: y += last via DVE add on a partition-replicated last tile.

Decoder exploits rank structure: h-side gates computed once per unique row
(1024), pe-side gates once per unique (s,c) col (512); combined per-s with
step-0 broadcast views, never materializing redundant matmuls.
"""
import numpy as np
import ml_dtypes

import concourse.bass as bass
import concourse.bacc as bacc
import concourse.mybir as mybir
from concourse import tile
from concourse.bass_utils import run_bass_kernel_spmd

bf16 = ml_dtypes.bfloat16
F32 = mybir.dt.float32
BF16 = mybir.dt.bfloat16
AF = mybir.ActivationFunctionType
ALU = mybir.AluOpType

B, SEQ, ENC = 128, 1024, 64
D, SEG = 512, 64
SNX = SEQ // SEG          # 16
PRED = 512
SNY = PRED // SEG         # 8
NCORES = 8
BL = B // NCORES          # 16 batches per core
R = BL * ENC              # 1024 rows per core
KC = D // 128             # 4 contraction chunks
G3 = 3 * D                # 1536 gate dims
MC = G3 // 128            # 12 gate chunks
FH = R // 512             # 2 free halves of the row range

# bias column map
BC_EMB, BC_RZ, BC_HN, BC_XN, BC_RES = 0, 4, 12, 16, 20
BC_RZD, BC_HND, BC_XND, BC_PRED = 24, 32, 36, 40

_PROGRAM = None
GP_SPLIT = False
SKIP_DEC = False


def _build_program():
    nc = bacc.Bacc("TRN2", target_bir_lowering=False, debug=False, num_devices=8)
    x_d = nc.dram_tensor("x", [BL, SEQ, ENC], F32, kind="ExternalInput")
    lastrow_d = nc.dram_tensor("lastrow", [1, R], F32, kind="ExternalInput")
    wemb_d = nc.dram_tensor("wemb", [65, D], BF16, kind="ExternalInput")
    wx_d = nc.dram_tensor("wx", [D, G3], BF16, kind="ExternalInput")
    wh_d = nc.dram_tensor("wh", [D, G3], BF16, kind="ExternalInput")
    wres_d = nc.dram_tensor("wres", [D, D], BF16, kind="ExternalInput")
    wxd_d = nc.dram_tensor("wxd", [D, G3], BF16, kind="ExternalInput")
    whd_d = nc.dram_tensor("whd", [D, G3], BF16, kind="ExternalInput")
    wpred_d = nc.dram_tensor("wpred", [D, SEG], BF16, kind="ExternalInput")
    pe_d = nc.dram_tensor("pe", [D, SNY * ENC], BF16, kind="ExternalInput")
    biases_d = nc.dram_tensor("biases", [128, 41], F32, kind="ExternalInput")
    o_d = nc.dram_tensor("o", [BL, PRED, ENC], F32, kind="ExternalOutput")

    with tile.TileContext(nc) as tc:
        with (
            tc.tile_pool(name="wp", bufs=1) as wp,
            tc.tile_pool(name="hp", bufs=2) as hp,
            tc.tile_pool(name="psum", bufs=8, space="PSUM") as pp,
        ):
            # ---- persistent weights ----
            def wload(name, dram, width):
                t = wp.tile([128, KC * width], BF16, tag=name)
                nc.sync.dma_start(t[:].rearrange("p (kc j) -> p kc j", kc=KC),
                                  dram[:].rearrange("(kc p) j -> p kc j", p=128))
                return t

            wemb = wp.tile([65, D], BF16, tag="wemb")
            nc.sync.dma_start(wemb[:], wemb_d[:])
            wx = wload("wx", wx_d, G3)
            wh = wload("wh", wh_d, G3)
            wres = wload("wres", wres_d, D)
            wxd = wload("wxd", wxd_d, G3)
            whd = wload("whd", whd_d, G3)
            wpred = wload("wpred", wpred_d, SEG)
            pet = wload("pet", pe_d, SNY * ENC)
            bia = wp.tile([128, 41], F32, tag="bia")
            nc.sync.dma_start(bia[:], biases_d[:])
            last64 = wp.tile([64, R], F32, tag="last64")
            nc.sync.dma_start(last64[:], lastrow_d[:].partition_broadcast(64))

            def wsl(w, kc, mc, width=G3):
                return w[:, kc * width + mc * 128: kc * width + mc * 128 + 128]

            # ---- initial state ----
            hT = [hp.tile([128, R], BF16, tag=f"h{i}", name=f"h{i}") for i in range(KC)]
            for i in range(KC):
                nc.vector.memset(hT[i][:], 0.0)

            with (
                tc.tile_pool(name="xs", bufs=3) as xsp,
                tc.tile_pool(name="emb", bufs=2) as embp,
                tc.tile_pool(name="gat", bufs=1) as gatp,
                tc.tile_pool(name="tmp", bufs=3) as tmpp,
            ):
                for t in range(SNX):
                    # -- load + cast x segment: xsT [65, R] (row 64 = last) --
                    xsf = xsp.tile([65, R], F32, tag="xsf")
                    nc.sync.dma_start(
                        xsf[0:64, :].rearrange("k (b c) -> k b c", b=BL),
                        x_d[:, t * SEG:(t + 1) * SEG, :].rearrange("b k c -> k b c"))
                    nc.sync.dma_start(xsf[64:65, :], lastrow_d[:])
                    xsb = xsp.tile([65, R], BF16, tag="xsb")
                    nc.vector.tensor_copy(xsb[:], xsf[:])

                    # -- embT = silu((x-last) @ W_emb^T + b) : [D, R] --
                    embT = embp.tile([128, KC * R], BF16, tag="embT")
                    for mc in range(KC):
                        for fh in range(FH):
                            ps = pp.tile([128, 512], F32, tag="ps")
                            nc.tensor.matmul(
                                ps[:], wemb[:, mc * 128:(mc + 1) * 128],
                                xsb[:, fh * 512:(fh + 1) * 512],
                                start=True, stop=True)
                            sg = tmpp.tile([128, 512], BF16, tag="sg")
                            nc.scalar.activation(sg[:], ps[:], AF.Sigmoid,
                                                 bias=bia[:, BC_EMB + mc: BC_EMB + mc + 1])
                            # silu = (ps + b_emb) * sigmoid
                            nc.vector.scalar_tensor_tensor(
                                embT[:, mc * R + fh * 512: mc * R + (fh + 1) * 512],
                                ps[:], bia[:, BC_EMB + mc: BC_EMB + mc + 1], sg[:],
                                ALU.add, ALU.mult)

                    def eT(mc, fh):
                        return embT[:, mc * R + fh * 512: mc * R + (fh + 1) * 512]

                    # -- gates --
                    rz = gatp.tile([128, 8 * R], BF16, tag="rz")   # r: 0..3, z: 4..7
                    nsb = gatp.tile([128, 4 * R], BF16, tag="nsb")
                    for fh in range(FH):
                        for mc in range(8):   # r and z chunks
                            ps = pp.tile([128, 512], F32, tag="ps")
                            nk = KC if t > 0 else 0   # h == 0 at t == 0
                            for kc in range(KC):
                                nc.tensor.matmul(ps[:], wsl(wx, kc, mc), eT(kc, fh),
                                                 start=(kc == 0),
                                                 stop=(nk == 0 and kc == KC - 1))
                            for kc in range(nk):
                                nc.tensor.matmul(ps[:], wsl(wh, kc, mc),
                                                 hT[kc][:, fh * 512:(fh + 1) * 512],
                                                 start=False, stop=(kc == nk - 1))
                            nc.scalar.activation(
                                rz[:, mc * R + fh * 512: mc * R + (fh + 1) * 512],
                                ps[:], AF.Sigmoid,
                                bias=bia[:, BC_RZ + mc: BC_RZ + mc + 1])
                        for mc in range(4):   # n chunks: x-side and h-side separate
                            psx = pp.tile([128, 512], F32, tag="ps")
                            for kc in range(KC):
                                nc.tensor.matmul(psx[:], wsl(wx, kc, 8 + mc), eT(kc, fh),
                                                 start=(kc == 0), stop=(kc == KC - 1))
                            # t1 = (psh + bhh_n) * r ; n = tanh(t1 + psx + bih_n)
                            t1 = tmpp.tile([128, 512], BF16, tag="t1")
                            if t > 0:
                                psh = pp.tile([128, 512], F32, tag="ps")
                                for kc in range(KC):
                                    nc.tensor.matmul(psh[:], wsl(wh, kc, 8 + mc),
                                                     hT[kc][:, fh * 512:(fh + 1) * 512],
                                                     start=(kc == 0), stop=(kc == KC - 1))
                                nc.vector.scalar_tensor_tensor(
                                    t1[:], psh[:], bia[:, BC_HN + mc: BC_HN + mc + 1],
                                    rz[:, mc * R + fh * 512: mc * R + (fh + 1) * 512],
                                    ALU.add, ALU.mult)
                            else:
                                # h == 0: t1 = bhh_n * r
                                nc.vector.tensor_scalar(
                                    t1[:],
                                    rz[:, mc * R + fh * 512: mc * R + (fh + 1) * 512],
                                    bia[:, BC_HN + mc: BC_HN + mc + 1], None,
                                    ALU.mult)
                            t2 = tmpp.tile([128, 512], BF16, tag="t2")
                            nc.vector.tensor_tensor(t2[:], psx[:], t1[:], ALU.add)
                            nc.scalar.activation(
                                nsb[:, mc * R + fh * 512: mc * R + (fh + 1) * 512],
                                t2[:], AF.Tanh,
                                bias=bia[:, BC_XN + mc: BC_XN + mc + 1])

                    # -- h_cell = n + z*(h - n) --
                    hc = gatp.tile([128, KC * R], BF16, tag="hc")
                    for mc in range(KC):
                        nsl = nsb[:, mc * R:(mc + 1) * R]
                        zsl = rz[:, (4 + mc) * R:(5 + mc) * R]
                        csl = hc[:, mc * R:(mc + 1) * R]
                        eng = nc.vector if (mc < 2 or not GP_SPLIT) else nc.gpsimd
                        if t > 0:
                            eng.tensor_tensor(csl, hT[mc][:], nsl, ALU.subtract)
                            eng.tensor_tensor(csl, csl, zsl, ALU.mult)
                            eng.tensor_tensor(csl, csl, nsl, ALU.add)
                        else:
                            # h == 0: hc = n - z*n
                            eng.tensor_tensor(csl, zsl, nsl, ALU.mult)
                            eng.tensor_tensor(csl, nsl, csl, ALU.subtract)
                    # -- h_new = embT + (hc @ resW^T + res_b) --
                    hT_new = [hp.tile([128, R], BF16, tag=f"h{i}", name=f"hn{i}") for i in range(KC)]
                    for fh in range(FH):
                        for mc in range(KC):
                            ps = pp.tile([128, 512], F32, tag="ps")
                            for kc in range(KC):
                                nc.tensor.matmul(
                                    ps[:], wsl(wres, kc, mc, D),
                                    hc[:, kc * R + fh * 512: kc * R + (fh + 1) * 512],
                                    start=(kc == 0), stop=(kc == KC - 1))
                            nc.vector.scalar_tensor_tensor(
                                hT_new[mc][:, fh * 512:(fh + 1) * 512],
                                ps[:], bia[:, BC_RES + mc: BC_RES + mc + 1],
                                eT(mc, fh), ALU.add, ALU.add)
                    hT = hT_new

            # ================= decoder =================
            if not SKIP_DEC:
                with (
                    tc.tile_pool(name="dg", bufs=1) as dgp,
                    tc.tile_pool(name="dw", bufs=2) as dwp,
                ):
                    # h-side gates for the 1024 unique rows: ghd [G3, R] bf16
                    ghd = dgp.tile([128, MC * R], BF16, tag="ghd")
                    for mc in range(MC):
                        for fh in range(FH):
                            ps = pp.tile([128, 512], F32, tag="ps")
                            for kc in range(KC):
                                nc.tensor.matmul(ps[:], wsl(whd, kc, mc),
                                                 hT[kc][:, fh * 512:(fh + 1) * 512],
                                                 start=(kc == 0), stop=(kc == KC - 1))
                            nc.scalar.copy(
                                ghd[:, mc * R + fh * 512: mc * R + (fh + 1) * 512], ps[:])
                    # pe-side gates for the 512 unique (s,c) cols: gxd [G3, 512]
                    gxd = dgp.tile([128, MC * 512], BF16, tag="gxd")
                    for mc in range(MC):
                        ps = pp.tile([128, 512], F32, tag="ps")
                        for kc in range(KC):
                            nc.tensor.matmul(ps[:], wsl(wxd, kc, mc),
                                             pet[:, kc * 512:(kc + 1) * 512],
                                             start=(kc == 0), stop=(kc == KC - 1))
                        nc.scalar.copy(gxd[:, mc * 512:(mc + 1) * 512], ps[:])

                    def gxv(mc, s):   # pe-side view for fixed s: broadcast over b
                        v = gxd[:, mc * 512 + s * ENC: mc * 512 + (s + 1) * ENC]
                        return v.unsqueeze(1).to_broadcast((128, BL, ENC))

                    for s in range(SNY):
                        rzd = dwp.tile([128, 8 * R], BF16, tag="rzd")
                        for mc in range(8):
                            u = dwp.tile([128, R], BF16, tag="u")
                            nc.vector.tensor_tensor(
                                u[:].rearrange("p (b c) -> p b c", b=BL),
                                ghd[:, mc * R:(mc + 1) * R]
                                .rearrange("p (b c) -> p b c", b=BL),
                                gxv(mc, s), ALU.add)
                            nc.scalar.activation(
                                rzd[:, mc * R:(mc + 1) * R], u[:], AF.Sigmoid,
                                bias=bia[:, BC_RZD + mc: BC_RZD + mc + 1])
                        nd = dwp.tile([128, 4 * R], BF16, tag="nd")
                        for mc in range(4):
                            # t1 = (ghd_n + gbhh_n) * r ; n = tanh(t1 + gx_n + gbih_n)
                            t1 = dwp.tile([128, R], BF16, tag="dt1")
                            nc.vector.scalar_tensor_tensor(
                                t1[:], ghd[:, (8 + mc) * R:(9 + mc) * R],
                                bia[:, BC_HND + mc: BC_HND + mc + 1],
                                rzd[:, mc * R:(mc + 1) * R], ALU.add, ALU.mult)
                            t2 = dwp.tile([128, R], BF16, tag="dt2")
                            nc.vector.tensor_tensor(
                                t2[:].rearrange("p (b c) -> p b c", b=BL),
                                t1[:].rearrange("p (b c) -> p b c", b=BL),
                                gxv(8 + mc, s), ALU.add)
                            nc.scalar.activation(
                                nd[:, mc * R:(mc + 1) * R], t2[:], AF.Tanh,
                                bias=bia[:, BC_XND + mc: BC_XND + mc + 1])
                        # hy = n + z*(h0d - n)
                        hy = dwp.tile([128, KC * R], BF16, tag="hy")
                        for mc in range(KC):
                            nsl = nd[:, mc * R:(mc + 1) * R]
                            zsl = rzd[:, (4 + mc) * R:(5 + mc) * R]
                            ysl = hy[:, mc * R:(mc + 1) * R]
                            eng = nc.vector if (mc < 2 or not GP_SPLIT) else nc.gpsimd
                            eng.tensor_tensor(ysl, hT[mc][:], nsl, ALU.subtract)
                            eng.tensor_tensor(ysl, ysl, zsl, ALU.mult)
                            eng.tensor_tensor(ysl, ysl, nsl, ALU.add)
                        # y = hy @ predW^T + pred_b + last
                        yt = dwp.tile([64, R], F32, tag="yt")
                        for q in range(FH):
                            ps = pp.tile([64, 512], F32, tag="ps")
                            for kc in range(KC):
                                nc.tensor.matmul(
                                    ps[:], wpred[:, kc * SEG:(kc + 1) * SEG],
                                    hy[:, kc * R + q * 512: kc * R + (q + 1) * 512],
                                    start=(kc == 0), stop=(kc == KC - 1))
                            nc.scalar.activation(yt[:, q * 512:(q + 1) * 512], ps[:],
                                                 AF.Identity,
                                                 bias=bia[0:64, BC_PRED: BC_PRED + 1])
                        nc.vector.tensor_tensor(yt[:], yt[:], last64[:], ALU.add)
                        # store: o[b, s*64+k, c] = yt[k, b*64 + c]
                        nc.sync.dma_start(
                            o_d[:, s * SEG:(s + 1) * SEG, :].rearrange("b k c -> k b c"),
                            yt[:].rearrange("k (b c) -> k b c", b=BL))
    nc.finalize()
    return nc


def _prep_host(inputs):
    f = lambda a: np.ascontiguousarray(a, dtype=np.float32)
    bfc = lambda a: np.ascontiguousarray(a).astype(bf16)
    W_emb = f(inputs["W_emb"])                      # (D, SEG)
    wemb = np.zeros((65, D), np.float32)
    wemb[0:64, :] = W_emb.T
    wemb[64, :] = -W_emb.sum(axis=1)
    Wih, Whh = f(inputs["cell_Wih"]), f(inputs["cell_Whh"])
    bih, bhh = f(inputs["cell_bih"]), f(inputs["cell_bhh"])
    resW, resb = f(inputs["res_W"]), f(inputs["res_b"])
    gWih, gWhh = f(inputs["gru_Wih"]), f(inputs["gru_Whh"])
    gbih, gbhh = f(inputs["gru_bih"]), f(inputs["gru_bhh"])
    predW, predb = f(inputs["pred_W"]), f(inputs["pred_b"])
    pos_emb, channel_emb = f(inputs["pos_emb"]), f(inputs["channel_emb"])

    pe = np.zeros((D, SNY * ENC), np.float32)       # cols j = s*64 + c
    half = D // 2
    pe[0:half, :] = np.repeat(pos_emb.T, ENC, axis=1)          # pos[s,:] per col
    pe[half:, :] = np.tile(channel_emb.T, (1, SNY))            # ch[c,:] per col

    biases = np.zeros((128, 41), np.float32)

    def put(col, vec):
        nch = len(vec) // 128 if len(vec) >= 128 else 1
        for i in range(nch):
            seg = vec[i * 128:(i + 1) * 128]
            biases[0:len(seg), col + i] = seg

    put(BC_EMB, f(inputs["b_emb"]))
    put(BC_RZ, (bih + bhh)[0:1024])
    put(BC_HN, bhh[1024:1536])
    put(BC_XN, bih[1024:1536])
    put(BC_RES, resb)
    put(BC_RZD, (gbih + gbhh)[0:1024])
    put(BC_HND, gbhh[1024:1536])
    put(BC_XND, gbih[1024:1536])
    put(BC_PRED, predb)

    return {
        "wemb": bfc(wemb),
        "wx": bfc(Wih.T), "wh": bfc(Whh.T), "wres": bfc(resW.T),
        "wxd": bfc(gWih.T), "whd": bfc(gWhh.T), "wpred": bfc(predW.T),
        "pe": bfc(pe), "biases": biases,
    }


def kernel(**inputs):
    global _PROGRAM
    if _PROGRAM is None:
        _PROGRAM = _build_program()
    nc = _PROGRAM
    shared = _prep_host(inputs)
    x = np.ascontiguousarray(inputs["x"], dtype=np.float32)
    in_maps = []
    for c in range(NCORES):
        xs = x[c * BL:(c + 1) * BL]
        m = dict(shared)
        m["x"] = xs
        m["lastrow"] = np.ascontiguousarray(xs[:, -1, :].reshape(1, R))
        in_maps.append(m)
    res = run_bass_kernel_spmd(nc, in_maps, list(range(NCORES)))
    out = np.concatenate([res.results[c]["o"] for c in range(NCORES)], axis=0)
    return out.astype(np.float32)

